# revision 14
# baseline (speedup 1.0000x reference)
"""Trainium2 Bass kernel for nn_BioinspiredNeuralNetwork (3-layer holographic MLP).

Math per layer i (complex):
    out = xc @ (Wr + i*Wi)
    act = sigmoid(beta_i * out.real) ** alpha_i
    xc  = act * out / |out| * mask_i     (mask: fixed PRNG key 42, host-precomputed)

Distribution: tensor-parallel over output columns across 8 cores (512 cols
each). Activations kept transposed [4096, 512] (neuron-major), replicated via
AllGather at each layer boundary.

Precision: 3-pass fp16 split matmuls (x = x_hi + x_lo, W = W_hi + W_lo, drop
lo*lo) — fp32-class error at full 1-cycle/row PE rate (HW-verified rms rel
err 2.6e-7 per 4096-deep matmul).

Complex product: 4 real products; the real part uses two PSUM banks
(P1 = xr@Wr, P2 = xi@Wi) combined as P1 - P2 in the epilogue, so no negated
weights are needed.

Overlap: each layer's 4 output m-tiles run in two phases (m01, m23). The
boundary AllGather is split in two halves: half A launches after phase 1's
epilogue and hides under phase 2's matmuls; half B hides under the next
layer's first-half chunks. All k-loops process half-A chunks first so
boundary reloads can overwrite them early.
"""

import numpy as np

import concourse.bass as bass
import concourse.mybir as mybir
import concourse.tile as tile
from concourse import bacc
from concourse.bass import ds, ts
from concourse.bass_utils import run_bass_kernel_spmd

AF = mybir.ActivationFunctionType

NCORES = 8
B = 512            # batch
D = 4096           # layer width
S = D // NCORES    # per-core output column shard (512)
KT = D // 128      # 32 k-chunks
MT = S // 128      # 4 m-tiles per shard
N_LAYERS = 3
N_CLUSTERS = 10
SPARSITY = 0.2

f32 = mybir.dt.float32
f16 = mybir.dt.float16

# AllGather half A covers rows 0:256 of every core's shard (chunks 4c, 4c+1),
# half B the rest. Every k-loop processes A-chunks first.
_CHUNKS_A = [4 * c + j for c in range(NCORES) for j in (0, 1)]
_CHUNKS_B = [4 * c + j for c in range(NCORES) for j in (2, 3)]
_CHUNKS = _CHUNKS_A + _CHUNKS_B


def _build(betas, alphas):
    nc = bacc.Bacc("TRN2", target_bir_lowering=False, debug=False,
                   num_devices=NCORES)

    xt = nc.dram_tensor("xt", [D, B], f32, kind="ExternalInput")
    # weight piece packs: [D, 2*S] fp16, cols 0:S = hi piece, S:2S = lo piece
    wrp = [nc.dram_tensor(f"w{l}rp", [D, 2 * S], f16, kind="ExternalInput")
           for l in range(N_LAYERS)]
    wip = [nc.dram_tensor(f"w{l}ip", [D, 2 * S], f16, kind="ExternalInput")
           for l in range(N_LAYERS)]
    msk = [nc.dram_tensor(f"mask{l}", [S, B], f16, kind="ExternalInput")
           for l in range(N_LAYERS)]
    outr = nc.dram_tensor("outr", [S, B], f32, kind="ExternalOutput")
    outi = nc.dram_tensor("outi", [S, B], f32, kind="ExternalOutput")

    # exchange buffers per boundary/half: rows [xr(2x128); xi(2x128)]
    agi = [[nc.dram_tensor(f"agi{b}{h}", [S, B], f32) for h in range(2)]
           for b in range(2)]
    ago = [[nc.dram_tensor(f"ago{b}{h}", [NCORES * S, B], f32,
                           addr_space="Shared") for h in range(2)]
           for b in range(2)]

    xt_r = xt.ap().rearrange("(n p) b -> n p b", p=128)
    wrp_r = [w.ap().rearrange("(n p) s -> n p s", p=128) for w in wrp]
    wip_r = [w.ap().rearrange("(n p) s -> n p s", p=128) for w in wip]
    msk_r = [m.ap().rearrange("(n p) b -> n p b", p=128) for m in msk]

    with tile.TileContext(nc) as tc:
        with (
            tc.tile_pool(name="xp", bufs=1) as xp,
            tc.tile_pool(name="wp", bufs=4) as wp,
            tc.tile_pool(name="rp", bufs=4) as rp,
            tc.tile_pool(name="mp", bufs=1) as mp,
            tc.tile_pool(name="ep", bufs=1) as ep,
            tc.tile_pool(name="ps", bufs=1, space="PSUM") as ps,
        ):
            # resident fp16 hi/lo pieces of transposed activations
            xrh = [xp.tile([128, B], f16, name=f"xrh{k}", tag=f"xrh{k}")
                   for k in range(KT)]
            xrl = [xp.tile([128, B], f16, name=f"xrl{k}", tag=f"xrl{k}")
                   for k in range(KT)]
            xih = [xp.tile([128, B], f16, name=f"xih{k}", tag=f"xih{k}")
                   for k in range(KT)]
            xil = [xp.tile([128, B], f16, name=f"xil{k}", tag=f"xil{k}")
                   for k in range(KT)]

            def load_split(k, src_ap, hi, lo, tag, eng):
                t = rp.tile([128, B], f32, name=tag, tag=tag)
                nc.sync.dma_start(out=t[:], in_=src_ap)
                eng.tensor_copy(hi[k][:], t[:])
                eng.tensor_sub(lo[k][:], t[:], hi[k][:])

            for i, k in enumerate(_CHUNKS):
                eng = nc.vector if i % 2 == 0 else nc.gpsimd
                load_split(k, xt_r[k], xrh, xrl, f"rl{i % 2}", eng)

            def mm_phase(l, pair, scope):
                """One m-pair phase of layer l's matmuls -> psum dict per m.
                l == 0: p1 = x@Wr, pi = x@Wi (real input).
                l >= 1: p1 = xr@Wr, p2 = xi@Wi, pi = xr@Wi + xi@Wr."""
                with nc.named_scope(scope):
                    p1 = {m: ps.tile([128, B], f32, name=f"p1s{i}", tag=f"p1s{i}")
                          for i, m in enumerate(pair)}
                    pi = {m: ps.tile([128, B], f32, name=f"pis{i}", tag=f"pis{i}")
                          for i, m in enumerate(pair)}
                    p2 = ({m: ps.tile([128, B], f32, name=f"p2s{i}",
                                      tag=f"p2s{i}") for i, m in enumerate(pair)}
                          if l > 0 else None)
                    for ki, k in enumerate(_CHUNKS):
                        first, last = ki == 0, ki == KT - 1
                        wr_t = wp.tile([128, 2 * S], f16, name="wr_t", tag="wr_t")
                        nc.sync.dma_start(out=wr_t[:], in_=wrp_r[l][k])
                        wi_t = wp.tile([128, 2 * S], f16, name="wi_t", tag="wi_t")
                        nc.sync.dma_start(out=wi_t[:], in_=wip_r[l][k])
                        mm = nc.tensor.matmul

                        def passes(p, wt, m, rh_t, rl_t, first, last):
                            hi = wt[:, ts(m, 128)]
                            lo = wt[:, ds(S + m * 128, 128)]
                            mm(p[:], lhsT=hi, rhs=rh_t[:], start=first,
                               stop=False)
                            mm(p[:], lhsT=hi, rhs=rl_t[:], start=False,
                               stop=False)
                            mm(p[:], lhsT=lo, rhs=rh_t[:], start=False,
                               stop=last)

                        for m in pair:
                            passes(p1[m], wr_t, m, xrh[k], xrl[k],
                                   first, last)
                            if l == 0:
                                passes(pi[m], wi_t, m, xrh[k], xrl[k],
                                       first, last)
                            else:
                                passes(p2[m], wi_t, m, xih[k], xil[k],
                                       first, last)
                                passes(pi[m], wi_t, m, xrh[k], xrl[k],
                                       first, False)
                                passes(pi[m], wr_t, m, xih[k], xil[k],
                                       False, last)
                    return p1, p2, pi

            def epilogue(l, pair, p1, p2, pi, mt, scope):
                """l == 0: out_r = p1, out_i = pi.
                l >= 1: out_r = p1 - p2, out_i = pi."""
                with nc.named_scope(scope):
                    for m in pair:
                        if l == 0:
                            orr_ap, oi_ap = p1[m][:], pi[m][:]
                        else:
                            c2 = ep.tile([128, B], f32, name="c2", tag="c2")
                            nc.scalar.copy(c2[:], p2[m][:])
                            orr = ep.tile([128, B], f32, name="orr", tag="orr")
                            nc.vector.tensor_sub(orr[:], p1[m][:], c2[:])
                            orr_ap, oi_ap = orr[:], pi[m][:]
                        act = ep.tile([128, B], f32, name="act", tag="act")
                        nc.scalar.activation(act[:], orr_ap, AF.Sigmoid,
                                             scale=float(betas[l]))
                        if abs(alphas[l] - 1.0) > 1e-12:
                            lg = ep.tile([128, B], f32, name="lg", tag="lg")
                            nc.scalar.activation(lg[:], act[:], AF.Ln)
                            nc.scalar.activation(act[:], lg[:], AF.Exp,
                                                 scale=float(alphas[l]))
                        u1 = ep.tile([128, B], f32, name="u1", tag="u1")
                        nc.scalar.activation(u1[:], orr_ap, AF.Square)
                        u2 = ep.tile([128, B], f32, name="u2", tag="u2")
                        nc.scalar.activation(u2[:], oi_ap, AF.Square)
                        u3 = ep.tile([128, B], f32, name="u3", tag="u3")
                        nc.vector.tensor_add(u3[:], u1[:], u2[:])
                        rin = ep.tile([128, B], f32, name="u1b", tag="u1b")
                        nc.vector.reciprocal(rin[:], u3[:])
                        q = ep.tile([128, B], f32, name="u2b", tag="u2b")
                        nc.scalar.activation(q[:], rin[:], AF.Sqrt)
                        f = ep.tile([128, B], f32, name="u3b", tag="u3b")
                        nc.vector.tensor_mul(f[:], act[:], q[:])
                        fm = ep.tile([128, B], f32, name="fm", tag="fm")
                        nc.vector.tensor_mul(fm[:], f[:], mt[m][:])
                        xnr = ep.tile([128, B], f32, name="xnr", tag="xnr")
                        nc.vector.tensor_mul(xnr[:], fm[:], orr_ap)
                        xni = ep.tile([128, B], f32, name="xni", tag="xni")
                        nc.vector.tensor_mul(xni[:], fm[:], oi_ap)
                        if l == N_LAYERS - 1:
                            nc.sync.dma_start(out=outr.ap()[ts(m, 128)],
                                              in_=xnr[:])
                            nc.sync.dma_start(out=outi.ap()[ts(m, 128)],
                                              in_=xni[:])
                        else:
                            h, j = divmod(m, 2)
                            dst = agi[l][h].ap()
                            nc.sync.dma_start(out=dst[ds(j * 128, 128)],
                                              in_=xnr[:])
                            nc.sync.dma_start(out=dst[ds(256 + j * 128, 128)],
                                              in_=xni[:])

            def allgather(l, h):
                nc.gpsimd.collective_compute(
                    "AllGather", mybir.AluOpType.bypass,
                    ins=[agi[l][h].ap().opt()],
                    outs=[ago[l][h].ap().opt()],
                    replica_groups=[list(range(NCORES))],
                )

            def reload_half(l, h, scope):
                with nc.named_scope(scope):
                    gao = ago[l][h].ap()
                    for c in range(NCORES):
                        for j in range(2):
                            k = 4 * c + 2 * h + j
                            base = c * S + j * 128
                            load_split(k, gao[ds(base, 128)], xrh, xrl,
                                       "rl0", nc.vector)
                            load_split(k, gao[ds(base + 256, 128)], xih, xil,
                                       "rl1", nc.gpsimd)

            for l in range(N_LAYERS):
                mt = [mp.tile([128, B], f16, name=f"mt{m}", tag=f"mt{m}")
                      for m in range(MT)]
                for m in range(MT):
                    nc.sync.dma_start(out=mt[m][:], in_=msk_r[l][m])

                p1, p2, pi = mm_phase(l, (0, 1), f"l{l}p1")
                epilogue(l, (0, 1), p1, p2, pi, mt, f"l{l}e1")
                if l < N_LAYERS - 1:
                    with nc.named_scope(f"x{l}a"):
                        allgather(l, 0)
                p1, p2, pi = mm_phase(l, (2, 3), f"l{l}p2")
                epilogue(l, (2, 3), p1, p2, pi, mt, f"l{l}e2")
                if l < N_LAYERS - 1:
                    with nc.named_scope(f"x{l}b"):
                        allgather(l, 1)
                    reload_half(l, 0, f"r{l}a")
                    reload_half(l, 1, f"r{l}b")

    nc.compile()
    return nc


_NC_CACHE: dict = {}
TRACE = False
LAST_RES = None


def _get_nc(betas, alphas):
    key = (tuple(betas), tuple(alphas))
    if key not in _NC_CACHE:
        _NC_CACHE[key] = _build(betas, alphas)
    return _NC_CACHE[key]


def _ctx_mask_host(layer_i, cw, asg, batch):
    """Exact replica of reference._ctx_mask — fixed PRNG key, depends on
    inputs only through cw (cluster weights) and asg (cluster assignment)."""
    import jax
    import jax.numpy as jnp

    cpu = jax.devices("cpu")[0]
    with jax.default_device(cpu):
        key = jax.random.fold_in(jax.random.key(42), layer_i)
        cw_j = jnp.asarray(cw)
        asg_j = jnp.asarray(asg)
        probs = jax.nn.softmax(cw_j)
        p = probs[asg_j] * SPARSITY
        n = asg.shape[0]
        k1, k2 = jax.random.split(key)
        bern = jax.random.uniform(k1, (batch, n)) < p
        u = jax.random.uniform(k2, (batch, n))
        segmax = jax.vmap(
            lambda ur: jax.ops.segment_max(ur, asg_j, num_segments=N_CLUSTERS)
        )(u)
        force = u >= segmax[:, asg_j]
        return np.asarray((bern | force).astype(jnp.float32))


def _split16(w):
    hi = w.astype(np.float16)
    lo = (w - hi.astype(np.float32)).astype(np.float16)
    return np.concatenate([hi, lo], axis=1)


def kernel(**inputs):
    x = np.asarray(inputs["x"], np.float32)
    betas = [float(v) for v in np.asarray(inputs["beta"], np.float32)]
    alphas = [float(v) for v in np.asarray(inputs["alpha"], np.float32)]

    nc = _get_nc(betas, alphas)

    xt = np.ascontiguousarray(x.T)
    masksT = [
        np.ascontiguousarray(
            _ctx_mask_host(
                l,
                np.asarray(inputs[f"cw{l}"], np.float32),
                np.asarray(inputs[f"asg{l}"]),
                x.shape[0],
            ).T
        ).astype(np.float16)
        for l in range(N_LAYERS)
    ]

    in_maps = []
    for c in range(NCORES):
        sl = slice(c * S, (c + 1) * S)
        m = {"xt": xt}
        for l in range(N_LAYERS):
            wr = np.asarray(inputs[f"W{l}r"], np.float32)[:, sl]
            wi = np.asarray(inputs[f"W{l}i"], np.float32)[:, sl]
            m[f"w{l}rp"] = _split16(wr)
            m[f"w{l}ip"] = _split16(wi)
            m[f"mask{l}"] = masksT[l][sl, :]
        in_maps.append(m)

    res = run_bass_kernel_spmd(nc, in_maps, core_ids=list(range(NCORES)),
                               trace=TRACE)
    global LAST_RES
    LAST_RES = res
    outr = np.concatenate([res.results[c]["outr"] for c in range(NCORES)], axis=0)
    outi = np.concatenate([res.results[c]["outi"] for c in range(NCORES)], axis=0)
    return (outr.T + 1j * outi.T).astype(np.complex64)


# revision 15
# speedup vs baseline: 1.0846x; 1.0846x over previous
"""Trainium2 Bass kernel for nn_BioinspiredNeuralNetwork (3-layer holographic MLP).

Math per layer i (complex):
    out = xc @ (Wr + i*Wi)
    act = sigmoid(beta_i * out.real) ** alpha_i
    xc  = act * out / |out| * mask_i     (mask: fixed PRNG key 42, host-precomputed)

Distribution: tensor-parallel over output columns across 8 cores (512 cols
each). Activations kept transposed [4096, 512] (neuron-major), replicated via
AllGather at each layer boundary.

Precision: 3-pass fp16 split matmuls (x = x_hi + x_lo, W = W_hi + W_lo, drop
lo*lo) — fp32-class error at full 1-cycle/row PE rate (HW-verified rms rel
err 2.6e-7 per 4096-deep matmul). The complex real part uses two PSUM banks
(P1 = xr@Wr, P2 = xi@Wi) combined as P1 - P2 in the epilogue.

Exchange: activations cross the AllGather already split into fp16 hi/lo
pieces (split once on the producing core — fp16 ALU casts are slow on DVE, so
receive-side splitting of all 32 chunks is avoided; reloads are pure DMAs
into the resident piece tiles). The input x is pre-split on the host.

Overlap: each layer's 4 output m-tiles run in two phases (m01, m23). The
boundary AllGather is split in two halves: half A launches after phase 1's
epilogue and hides under phase 2's matmuls; half B hides under the next
layer's first-half chunks. All k-loops process half-A chunks first so
boundary reloads can overwrite them early.
"""

import numpy as np

import concourse.bass as bass
import concourse.mybir as mybir
import concourse.tile as tile
from concourse import bacc
from concourse.bass import ds, ts
from concourse.bass_utils import run_bass_kernel_spmd

AF = mybir.ActivationFunctionType

NCORES = 8
B = 512            # batch
D = 4096           # layer width
S = D // NCORES    # per-core output column shard (512)
KT = D // 128      # 32 k-chunks
MT = S // 128      # 4 m-tiles per shard
N_LAYERS = 3
N_CLUSTERS = 10
SPARSITY = 0.2

f32 = mybir.dt.float32
f16 = mybir.dt.float16

# AllGather half A covers rows 0:256 of every core's shard (chunks 4c, 4c+1),
# half B the rest. Every k-loop processes A-chunks first.
_CHUNKS_A = [4 * c + j for c in range(NCORES) for j in (0, 1)]
_CHUNKS_B = [4 * c + j for c in range(NCORES) for j in (2, 3)]
_CHUNKS = _CHUNKS_A + _CHUNKS_B


def _build(betas, alphas):
    nc = bacc.Bacc("TRN2", target_bir_lowering=False, debug=False,
                   num_devices=NCORES)

    # input x^T pre-split into fp16 hi/lo pieces on the host
    xth = nc.dram_tensor("xth", [D, B], f16, kind="ExternalInput")
    xtl = nc.dram_tensor("xtl", [D, B], f16, kind="ExternalInput")
    # weight piece packs: [D, 2*S] fp16, cols 0:S = hi piece, S:2S = lo piece
    wrp = [nc.dram_tensor(f"w{l}rp", [D, 2 * S], f16, kind="ExternalInput")
           for l in range(N_LAYERS)]
    wip = [nc.dram_tensor(f"w{l}ip", [D, 2 * S], f16, kind="ExternalInput")
           for l in range(N_LAYERS)]
    msk = [nc.dram_tensor(f"mask{l}", [S, B], f16, kind="ExternalInput")
           for l in range(N_LAYERS)]
    outr = nc.dram_tensor("outr", [S, B], f32, kind="ExternalOutput")
    outi = nc.dram_tensor("outi", [S, B], f32, kind="ExternalOutput")

    # exchange buffers per boundary/half, fp16 pieces:
    # rows [xrh(2x128); xrl(2x128); xih(2x128); xil(2x128)]
    agi = [[nc.dram_tensor(f"agi{b}{h}", [2 * S, B], f16) for h in range(2)]
           for b in range(2)]
    ago = [[nc.dram_tensor(f"ago{b}{h}", [NCORES * 2 * S, B], f16,
                           addr_space="Shared") for h in range(2)]
           for b in range(2)]

    xth_r = xth.ap().rearrange("(n p) b -> n p b", p=128)
    xtl_r = xtl.ap().rearrange("(n p) b -> n p b", p=128)
    wrp_r = [w.ap().rearrange("(n p) s -> n p s", p=128) for w in wrp]
    wip_r = [w.ap().rearrange("(n p) s -> n p s", p=128) for w in wip]
    msk_r = [m.ap().rearrange("(n p) b -> n p b", p=128) for m in msk]

    with tile.TileContext(nc) as tc:
        with (
            tc.tile_pool(name="xp", bufs=1) as xp,
            tc.tile_pool(name="wp", bufs=4) as wp,
            tc.tile_pool(name="mp", bufs=1) as mp,
            tc.tile_pool(name="ep", bufs=1) as ep,
            tc.tile_pool(name="ps", bufs=1, space="PSUM") as ps,
        ):
            # resident fp16 hi/lo pieces of transposed activations
            xrh = [xp.tile([128, B], f16, name=f"xrh{k}", tag=f"xrh{k}")
                   for k in range(KT)]
            xrl = [xp.tile([128, B], f16, name=f"xrl{k}", tag=f"xrl{k}")
                   for k in range(KT)]
            xih = [xp.tile([128, B], f16, name=f"xih{k}", tag=f"xih{k}")
                   for k in range(KT)]
            xil = [xp.tile([128, B], f16, name=f"xil{k}", tag=f"xil{k}")
                   for k in range(KT)]

            for k in _CHUNKS:
                nc.sync.dma_start(out=xrh[k][:], in_=xth_r[k])
                nc.sync.dma_start(out=xrl[k][:], in_=xtl_r[k])

            def mm_phase(l, pair, scope):
                """One m-pair phase of layer l's matmuls -> psum dict per m.
                l == 0: p1 = x@Wr, pi = x@Wi (real input).
                l >= 1: p1 = xr@Wr, p2 = xi@Wi, pi = xr@Wi + xi@Wr."""
                with nc.named_scope(scope):
                    p1 = {m: ps.tile([128, B], f32, name=f"p1s{i}", tag=f"p1s{i}")
                          for i, m in enumerate(pair)}
                    pi = {m: ps.tile([128, B], f32, name=f"pis{i}", tag=f"pis{i}")
                          for i, m in enumerate(pair)}
                    p2 = ({m: ps.tile([128, B], f32, name=f"p2s{i}",
                                      tag=f"p2s{i}") for i, m in enumerate(pair)}
                          if l > 0 else None)
                    for ki, k in enumerate(_CHUNKS):
                        first, last = ki == 0, ki == KT - 1
                        wr_t = wp.tile([128, 2 * S], f16, name="wr_t", tag="wr_t")
                        nc.sync.dma_start(out=wr_t[:], in_=wrp_r[l][k])
                        wi_t = wp.tile([128, 2 * S], f16, name="wi_t", tag="wi_t")
                        nc.sync.dma_start(out=wi_t[:], in_=wip_r[l][k])
                        mm = nc.tensor.matmul

                        def passes(p, wt, m, rh_t, rl_t, first, last):
                            hi = wt[:, ts(m, 128)]
                            lo = wt[:, ds(S + m * 128, 128)]
                            mm(p[:], lhsT=hi, rhs=rh_t[:], start=first,
                               stop=False)
                            mm(p[:], lhsT=hi, rhs=rl_t[:], start=False,
                               stop=False)
                            mm(p[:], lhsT=lo, rhs=rh_t[:], start=False,
                               stop=last)

                        for m in pair:
                            passes(p1[m], wr_t, m, xrh[k], xrl[k],
                                   first, last)
                            if l == 0:
                                passes(pi[m], wi_t, m, xrh[k], xrl[k],
                                       first, last)
                            else:
                                passes(p2[m], wi_t, m, xih[k], xil[k],
                                       first, last)
                                passes(pi[m], wi_t, m, xrh[k], xrl[k],
                                       first, False)
                                passes(pi[m], wr_t, m, xih[k], xil[k],
                                       False, last)
                    return p1, p2, pi

            def epilogue(l, pair, p1, p2, pi, mt, scope):
                """l == 0: out_r = p1, out_i = pi.  l >= 1: out_r = p1 - p2.
                For boundary layers, split xnr/xni into fp16 pieces here
                (producer-side) and stage them for the AllGather."""
                with nc.named_scope(scope):
                    for m in pair:
                        if l == 0:
                            orr_ap, oi_ap = p1[m][:], pi[m][:]
                        else:
                            c2 = ep.tile([128, B], f32, name="c2", tag="c2")
                            nc.scalar.copy(c2[:], p2[m][:])
                            orr = ep.tile([128, B], f32, name="orr", tag="orr")
                            nc.vector.tensor_sub(orr[:], p1[m][:], c2[:])
                            orr_ap, oi_ap = orr[:], pi[m][:]
                        act = ep.tile([128, B], f32, name="act", tag="act")
                        nc.scalar.activation(act[:], orr_ap, AF.Sigmoid,
                                             scale=float(betas[l]))
                        if abs(alphas[l] - 1.0) > 1e-12:
                            lg = ep.tile([128, B], f32, name="lg", tag="lg")
                            nc.scalar.activation(lg[:], act[:], AF.Ln)
                            nc.scalar.activation(act[:], lg[:], AF.Exp,
                                                 scale=float(alphas[l]))
                        u1 = ep.tile([128, B], f32, name="u1", tag="u1")
                        nc.scalar.activation(u1[:], orr_ap, AF.Square)
                        u2 = ep.tile([128, B], f32, name="u2", tag="u2")
                        nc.scalar.activation(u2[:], oi_ap, AF.Square)
                        u3 = ep.tile([128, B], f32, name="u3", tag="u3")
                        nc.vector.tensor_add(u3[:], u1[:], u2[:])
                        rin = ep.tile([128, B], f32, name="u1b", tag="u1b")
                        nc.vector.reciprocal(rin[:], u3[:])
                        q = ep.tile([128, B], f32, name="u2b", tag="u2b")
                        nc.scalar.activation(q[:], rin[:], AF.Sqrt)
                        f = ep.tile([128, B], f32, name="u3b", tag="u3b")
                        nc.vector.tensor_mul(f[:], act[:], q[:])
                        fm = ep.tile([128, B], f32, name="fm", tag="fm")
                        nc.vector.tensor_mul(fm[:], f[:], mt[m][:])
                        xnr = ep.tile([128, B], f32, name="xnr", tag="xnr")
                        nc.vector.tensor_mul(xnr[:], fm[:], orr_ap)
                        xni = ep.tile([128, B], f32, name="xni", tag="xni")
                        nc.vector.tensor_mul(xni[:], fm[:], oi_ap)
                        if l == N_LAYERS - 1:
                            nc.sync.dma_start(out=outr.ap()[ts(m, 128)],
                                              in_=xnr[:])
                            nc.sync.dma_start(out=outi.ap()[ts(m, 128)],
                                              in_=xni[:])
                        else:
                            # producer-side fp16 split (DVE for r, GpSimd for i)
                            nrh = ep.tile([128, B], f16, name="nrh", tag="nrh")
                            nc.vector.tensor_copy(nrh[:], xnr[:])
                            nrl = ep.tile([128, B], f16, name="nrl", tag="nrl")
                            nc.vector.tensor_sub(nrl[:], xnr[:], nrh[:])
                            nih = ep.tile([128, B], f16, name="nih", tag="nih")
                            nc.gpsimd.tensor_copy(nih[:], xni[:])
                            nil_ = ep.tile([128, B], f16, name="nil", tag="nil")
                            nc.gpsimd.tensor_sub(nil_[:], xni[:], nih[:])
                            h, j = divmod(m, 2)
                            dst = agi[l][h].ap()
                            for sec, t in enumerate((nrh, nrl, nih, nil_)):
                                nc.sync.dma_start(
                                    out=dst[ds(sec * 256 + j * 128, 128)],
                                    in_=t[:])

            def allgather(l, h):
                nc.gpsimd.collective_compute(
                    "AllGather", mybir.AluOpType.bypass,
                    ins=[agi[l][h].ap().opt()],
                    outs=[ago[l][h].ap().opt()],
                    replica_groups=[list(range(NCORES))],
                )

            def reload_half(l, h, scope):
                """Pure DMAs: gathered fp16 pieces -> resident piece tiles."""
                with nc.named_scope(scope):
                    gao = ago[l][h].ap()
                    for c in range(NCORES):
                        for j in range(2):
                            k = 4 * c + 2 * h + j
                            base = c * 2 * S + j * 128
                            for sec, tiles in enumerate((xrh, xrl, xih, xil)):
                                nc.sync.dma_start(
                                    out=tiles[k][:],
                                    in_=gao[ds(base + sec * 256, 128)])

            for l in range(N_LAYERS):
                mt = [mp.tile([128, B], f16, name=f"mt{m}", tag=f"mt{m}")
                      for m in range(MT)]
                for m in range(MT):
                    nc.sync.dma_start(out=mt[m][:], in_=msk_r[l][m])

                p1, p2, pi = mm_phase(l, (0, 1), f"l{l}p1")
                epilogue(l, (0, 1), p1, p2, pi, mt, f"l{l}e1")
                if l < N_LAYERS - 1:
                    with nc.named_scope(f"x{l}a"):
                        allgather(l, 0)
                p1, p2, pi = mm_phase(l, (2, 3), f"l{l}p2")
                epilogue(l, (2, 3), p1, p2, pi, mt, f"l{l}e2")
                if l < N_LAYERS - 1:
                    with nc.named_scope(f"x{l}b"):
                        allgather(l, 1)
                    reload_half(l, 0, f"r{l}a")
                    reload_half(l, 1, f"r{l}b")

    nc.compile()
    return nc


_NC_CACHE: dict = {}
TRACE = False
LAST_RES = None


def _get_nc(betas, alphas):
    key = (tuple(betas), tuple(alphas))
    if key not in _NC_CACHE:
        _NC_CACHE[key] = _build(betas, alphas)
    return _NC_CACHE[key]


def _ctx_mask_host(layer_i, cw, asg, batch):
    """Exact replica of reference._ctx_mask — fixed PRNG key, depends on
    inputs only through cw (cluster weights) and asg (cluster assignment)."""
    import jax
    import jax.numpy as jnp

    cpu = jax.devices("cpu")[0]
    with jax.default_device(cpu):
        key = jax.random.fold_in(jax.random.key(42), layer_i)
        cw_j = jnp.asarray(cw)
        asg_j = jnp.asarray(asg)
        probs = jax.nn.softmax(cw_j)
        p = probs[asg_j] * SPARSITY
        n = asg.shape[0]
        k1, k2 = jax.random.split(key)
        bern = jax.random.uniform(k1, (batch, n)) < p
        u = jax.random.uniform(k2, (batch, n))
        segmax = jax.vmap(
            lambda ur: jax.ops.segment_max(ur, asg_j, num_segments=N_CLUSTERS)
        )(u)
        force = u >= segmax[:, asg_j]
        return np.asarray((bern | force).astype(jnp.float32))


def _split16(w):
    hi = w.astype(np.float16)
    lo = (w - hi.astype(np.float32)).astype(np.float16)
    return np.concatenate([hi, lo], axis=1)


def kernel(**inputs):
    x = np.asarray(inputs["x"], np.float32)
    betas = [float(v) for v in np.asarray(inputs["beta"], np.float32)]
    alphas = [float(v) for v in np.asarray(inputs["alpha"], np.float32)]

    nc = _get_nc(betas, alphas)

    xt = np.ascontiguousarray(x.T)
    xth = xt.astype(np.float16)
    xtl = (xt - xth.astype(np.float32)).astype(np.float16)
    masksT = [
        np.ascontiguousarray(
            _ctx_mask_host(
                l,
                np.asarray(inputs[f"cw{l}"], np.float32),
                np.asarray(inputs[f"asg{l}"]),
                x.shape[0],
            ).T
        ).astype(np.float16)
        for l in range(N_LAYERS)
    ]

    in_maps = []
    for c in range(NCORES):
        sl = slice(c * S, (c + 1) * S)
        m = {"xth": xth, "xtl": xtl}
        for l in range(N_LAYERS):
            wr = np.asarray(inputs[f"W{l}r"], np.float32)[:, sl]
            wi = np.asarray(inputs[f"W{l}i"], np.float32)[:, sl]
            m[f"w{l}rp"] = _split16(wr)
            m[f"w{l}ip"] = _split16(wi)
            m[f"mask{l}"] = masksT[l][sl, :]
        in_maps.append(m)

    res = run_bass_kernel_spmd(nc, in_maps, core_ids=list(range(NCORES)),
                               trace=TRACE)
    global LAST_RES
    LAST_RES = res
    outr = np.concatenate([res.results[c]["outr"] for c in range(NCORES)], axis=0)
    outi = np.concatenate([res.results[c]["outi"] for c in range(NCORES)], axis=0)
    return (outr.T + 1j * outi.T).astype(np.complex64)


# revision 23
# speedup vs baseline: 1.1496x; 1.0600x over previous
"""Trainium2 Bass kernel for nn_BioinspiredNeuralNetwork (3-layer holographic MLP).

Math per layer i (complex):
    out = xc @ (Wr + i*Wi)
    act = sigmoid(beta_i * out.real) ** alpha_i
    xc  = act * out / |out| * mask_i     (mask: fixed PRNG key 42, host-precomputed)

Distribution: tensor-parallel over output columns across 8 cores (512 cols
each). Activations kept transposed [4096, 512] (neuron-major), replicated via
AllGather at each layer boundary.

Precision: 3-pass fp16 split matmuls (x = x_hi + x_lo, W = W_hi + W_lo, drop
lo*lo) — fp32-class error at full 1-cycle/row PE rate (HW-verified rms rel
err 2.6e-7 per 4096-deep matmul). The complex real part uses two PSUM banks
(P1 = xr@Wr, P2 = xi@Wi) combined as P1 - P2 in the epilogue.

Exchange: activations cross the AllGather already split into fp16 hi/lo
pieces (split once on the producing core — fp16 ALU casts are slow on DVE, so
receive-side splitting of all 32 chunks is avoided; reloads are pure DMAs
into the resident piece tiles). The input x is pre-split on the host.

Overlap: each layer's 4 output m-tiles run in two phases (m01, m23). The
boundary AllGather is split in two halves: half A launches after phase 1's
epilogue and hides under phase 2's matmuls; half B hides under the next
layer's first-half chunks. All k-loops process half-A chunks first so
boundary reloads can overwrite them early.
"""

import numpy as np

import concourse.bass as bass
import concourse.mybir as mybir
import concourse.tile as tile
from concourse import bacc
from concourse.bass import ds, ts
from concourse.bass_utils import run_bass_kernel_spmd

AF = mybir.ActivationFunctionType

NCORES = 8
B = 512            # batch
D = 4096           # layer width
S = D // NCORES    # per-core output column shard (512)
KT = D // 128      # 32 k-chunks
MT = S // 128      # 4 m-tiles per shard
N_LAYERS = 3
N_CLUSTERS = 10
SPARSITY = 0.2

f32 = mybir.dt.float32
f16 = mybir.dt.float16

# The boundary AllGather is split in four quarters, one per m-tile: quarter q
# carries shard rows q*128:(q+1)*128 of every core, i.e. chunks {4c+q}. Every
# k-loop processes chunks in quarter order so reload DMAs can overwrite the
# earliest-read chunks while later quarters are still gathering.
_CHUNKS = [4 * c + q for q in range(4) for c in range(NCORES)]


def _build(betas, alphas):
    nc = bacc.Bacc("TRN2", target_bir_lowering=False, debug=False,
                   num_devices=NCORES)

    # input x^T pre-split into fp16 hi/lo pieces on the host
    xth = nc.dram_tensor("xth", [D, B], f16, kind="ExternalInput")
    xtl = nc.dram_tensor("xtl", [D, B], f16, kind="ExternalInput")
    # weight piece packs: [D, 2*S] fp16, cols 0:S = hi piece, S:2S = lo piece
    wrp = [nc.dram_tensor(f"w{l}rp", [D, 2 * S], f16, kind="ExternalInput")
           for l in range(N_LAYERS)]
    wip = [nc.dram_tensor(f"w{l}ip", [D, 2 * S], f16, kind="ExternalInput")
           for l in range(N_LAYERS)]
    msk = [nc.dram_tensor(f"mask{l}", [S, B], f16, kind="ExternalInput")
           for l in range(N_LAYERS)]
    outr = nc.dram_tensor("outr", [S, B], f32, kind="ExternalOutput")
    outi = nc.dram_tensor("outi", [S, B], f32, kind="ExternalOutput")

    # exchange buffers per boundary/quarter, fp16 pieces:
    # rows [xrh(128); xrl(128); xih(128); xil(128)]
    agi = [[nc.dram_tensor(f"agi{b}{q}", [S, B], f16) for q in range(4)]
           for b in range(2)]
    ago = [[nc.dram_tensor(f"ago{b}{q}", [NCORES * S, B], f16,
                           addr_space="Shared") for q in range(4)]
           for b in range(2)]

    xth_r = xth.ap().rearrange("(n p) b -> n p b", p=128)
    xtl_r = xtl.ap().rearrange("(n p) b -> n p b", p=128)
    wrp_r = [w.ap().rearrange("(n p) s -> n p s", p=128) for w in wrp]
    wip_r = [w.ap().rearrange("(n p) s -> n p s", p=128) for w in wip]
    msk_r = [m.ap().rearrange("(n p) b -> n p b", p=128) for m in msk]

    with tile.TileContext(nc) as tc:
        with (
            tc.tile_pool(name="xp", bufs=1) as xp,
            tc.tile_pool(name="wp", bufs=4) as wp,
            tc.tile_pool(name="mp", bufs=1) as mp,
            tc.tile_pool(name="ep", bufs=1) as ep,
            tc.tile_pool(name="ps", bufs=1, space="PSUM") as ps,
        ):
            # resident fp16 hi/lo pieces of transposed activations
            xrh = [xp.tile([128, B], f16, name=f"xrh{k}", tag=f"xrh{k}")
                   for k in range(KT)]
            xrl = [xp.tile([128, B], f16, name=f"xrl{k}", tag=f"xrl{k}")
                   for k in range(KT)]
            xih = [xp.tile([128, B], f16, name=f"xih{k}", tag=f"xih{k}")
                   for k in range(KT)]
            xil = [xp.tile([128, B], f16, name=f"xil{k}", tag=f"xil{k}")
                   for k in range(KT)]

            def mm_phase(l, pair, scope, load_x=False):
                """One m-pair phase of layer l's matmuls -> psum dict per m.
                l == 0: p1 = x@Wr, pi = x@Wi (real input).
                l >= 1: p1 = xr@Wr, p2 = xi@Wi, pi = xr@Wi + xi@Wr."""
                with nc.named_scope(scope):
                    p1 = {m: ps.tile([128, B], f32, name=f"p1s{i}", tag=f"p1s{i}")
                          for i, m in enumerate(pair)}
                    pi = {m: ps.tile([128, B], f32, name=f"pis{i}", tag=f"pis{i}")
                          for i, m in enumerate(pair)}
                    p2 = ({m: ps.tile([128, B], f32, name=f"p2s{i}",
                                      tag=f"p2s{i}") for i, m in enumerate(pair)}
                          if l > 0 else None)
                    for ki, k in enumerate(_CHUNKS):
                        first, last = ki == 0, ki == KT - 1
                        if load_x:
                            nc.sync.dma_start(out=xrh[k][:], in_=xth_r[k])
                            nc.sync.dma_start(out=xrl[k][:], in_=xtl_r[k])
                        wr_t = wp.tile([128, 2 * S], f16, name="wr_t", tag="wr_t")
                        nc.sync.dma_start(out=wr_t[:], in_=wrp_r[l][k])
                        wi_t = wp.tile([128, 2 * S], f16, name="wi_t", tag="wi_t")
                        nc.sync.dma_start(out=wi_t[:], in_=wip_r[l][k])
                        mm = nc.tensor.matmul

                        def passes(p, wt, m, rh_t, rl_t, first, last):
                            hi = wt[:, ts(m, 128)]
                            lo = wt[:, ds(S + m * 128, 128)]
                            mm(p[:], lhsT=hi, rhs=rh_t[:], start=first,
                               stop=False)
                            mm(p[:], lhsT=hi, rhs=rl_t[:], start=False,
                               stop=False)
                            mm(p[:], lhsT=lo, rhs=rh_t[:], start=False,
                               stop=last)

                        for m in pair:
                            passes(p1[m], wr_t, m, xrh[k], xrl[k],
                                   first, last)
                            if l == 0:
                                passes(pi[m], wi_t, m, xrh[k], xrl[k],
                                       first, last)
                            else:
                                passes(p2[m], wi_t, m, xih[k], xil[k],
                                       first, last)
                                passes(pi[m], wi_t, m, xrh[k], xrl[k],
                                       first, False)
                                passes(pi[m], wr_t, m, xih[k], xil[k],
                                       False, last)
                    return p1, p2, pi

            def epilogue(l, pair, p1, p2, pi, mt, scope):
                """l == 0: out_r = p1, out_i = pi.  l >= 1: out_r = p1 - p2.
                For boundary layers, split xnr/xni into fp16 pieces here
                (producer-side) and stage them for the AllGather."""
                with nc.named_scope(scope):
                    for m in pair:
                        if l == 0:
                            orr_ap, oi_ap = p1[m][:], pi[m][:]
                        else:
                            c2 = ep.tile([128, B], f32, name="c2", tag="c2")
                            nc.scalar.copy(c2[:], p2[m][:])
                            orr = ep.tile([128, B], f32, name="orr", tag="orr")
                            nc.vector.tensor_sub(orr[:], p1[m][:], c2[:])
                            orr_ap, oi_ap = orr[:], pi[m][:]
                        act = ep.tile([128, B], f32, name="act", tag="act")
                        nc.scalar.activation(act[:], orr_ap, AF.Sigmoid,
                                             scale=float(betas[l]))
                        if abs(alphas[l] - 1.0) > 1e-12:
                            lg = ep.tile([128, B], f32, name="lg", tag="lg")
                            nc.scalar.activation(lg[:], act[:], AF.Ln)
                            nc.scalar.activation(act[:], lg[:], AF.Exp,
                                                 scale=float(alphas[l]))
                        u1 = ep.tile([128, B], f32, name="u1", tag="u1")
                        nc.scalar.activation(u1[:], orr_ap, AF.Square)
                        u2 = ep.tile([128, B], f32, name="u2", tag="u2")
                        nc.scalar.activation(u2[:], oi_ap, AF.Square)
                        u3 = ep.tile([128, B], f32, name="u3", tag="u3")
                        nc.vector.tensor_add(u3[:], u1[:], u2[:])
                        rin = ep.tile([128, B], f32, name="u1b", tag="u1b")
                        nc.vector.reciprocal(rin[:], u3[:])
                        q = ep.tile([128, B], f32, name="u2b", tag="u2b")
                        nc.scalar.activation(q[:], rin[:], AF.Sqrt)
                        f = ep.tile([128, B], f32, name="u3b", tag="u3b")
                        nc.vector.tensor_mul(f[:], act[:], q[:])
                        fm = ep.tile([128, B], f32, name="fm", tag="fm")
                        nc.vector.tensor_mul(fm[:], f[:], mt[m][:])
                        xnr = ep.tile([128, B], f32, name="xnr", tag="xnr")
                        nc.vector.tensor_mul(xnr[:], fm[:], orr_ap)
                        xni = ep.tile([128, B], f32, name="xni", tag="xni")
                        nc.vector.tensor_mul(xni[:], fm[:], oi_ap)
                        if l == N_LAYERS - 1:
                            nc.sync.dma_start(out=outr.ap()[ts(m, 128)],
                                              in_=xnr[:])
                            nc.sync.dma_start(out=outi.ap()[ts(m, 128)],
                                              in_=xni[:])
                        else:
                            # producer-side fp16 split (DVE for r, GpSimd for i)
                            nrh = ep.tile([128, B], f16, name="nrh", tag="nrh")
                            nc.vector.tensor_copy(nrh[:], xnr[:])
                            nrl = ep.tile([128, B], f16, name="nrl", tag="nrl")
                            nc.vector.tensor_sub(nrl[:], xnr[:], nrh[:])
                            nih = ep.tile([128, B], f16, name="nih", tag="nih")
                            nc.gpsimd.tensor_copy(nih[:], xni[:])
                            nil_ = ep.tile([128, B], f16, name="nil", tag="nil")
                            nc.gpsimd.tensor_sub(nil_[:], xni[:], nih[:])
                            dst = agi[l][m].ap()
                            for sec, t in enumerate((nrh, nrl, nih, nil_)):
                                nc.sync.dma_start(
                                    out=dst[ds(sec * 128, 128)], in_=t[:])
                            with nc.named_scope(f"x{l}q{m}"):
                                allgather(l, m)

            def allgather(l, q):
                nc.gpsimd.collective_compute(
                    "AllGather", mybir.AluOpType.bypass,
                    ins=[agi[l][q].ap().opt()],
                    outs=[ago[l][q].ap().opt()],
                    replica_groups=[list(range(NCORES))],
                )

            def reload_quarter(l, q, scope):
                """Pure DMAs: gathered fp16 pieces -> resident piece tiles."""
                with nc.named_scope(scope):
                    gao = ago[l][q].ap()
                    for c in range(NCORES):
                        k = 4 * c + q
                        base = c * S
                        for sec, tiles in enumerate((xrh, xrl, xih, xil)):
                            nc.sync.dma_start(
                                out=tiles[k][:],
                                in_=gao[ds(base + sec * 128, 128)])

            for l in range(N_LAYERS):
                mt = [mp.tile([128, B], f16, name=f"mt{m}", tag=f"mt{m}")
                      for m in range(MT)]
                for m in range(MT):
                    nc.sync.dma_start(out=mt[m][:], in_=msk_r[l][m])

                p1, p2, pi = mm_phase(l, (0, 1), f"l{l}p1", load_x=(l == 0))
                epilogue(l, (0, 1), p1, p2, pi, mt, f"l{l}e1")
                p1, p2, pi = mm_phase(l, (2, 3), f"l{l}p2")
                epilogue(l, (2, 3), p1, p2, pi, mt, f"l{l}e2")
                if l < N_LAYERS - 1:
                    for q in range(4):
                        reload_quarter(l, q, f"r{l}q{q}")

    nc.compile()
    return nc


_NC_CACHE: dict = {}
TRACE = False
LAST_RES = None


def _get_nc(betas, alphas):
    key = (tuple(betas), tuple(alphas))
    if key not in _NC_CACHE:
        _NC_CACHE[key] = _build(betas, alphas)
    return _NC_CACHE[key]


def _ctx_mask_host(layer_i, cw, asg, batch):
    """Exact replica of reference._ctx_mask — fixed PRNG key, depends on
    inputs only through cw (cluster weights) and asg (cluster assignment)."""
    import jax
    import jax.numpy as jnp

    cpu = jax.devices("cpu")[0]
    with jax.default_device(cpu):
        key = jax.random.fold_in(jax.random.key(42), layer_i)
        cw_j = jnp.asarray(cw)
        asg_j = jnp.asarray(asg)
        probs = jax.nn.softmax(cw_j)
        p = probs[asg_j] * SPARSITY
        n = asg.shape[0]
        k1, k2 = jax.random.split(key)
        bern = jax.random.uniform(k1, (batch, n)) < p
        u = jax.random.uniform(k2, (batch, n))
        segmax = jax.vmap(
            lambda ur: jax.ops.segment_max(ur, asg_j, num_segments=N_CLUSTERS)
        )(u)
        force = u >= segmax[:, asg_j]
        return np.asarray((bern | force).astype(jnp.float32))


def _split16(w):
    hi = w.astype(np.float16)
    lo = (w - hi.astype(np.float32)).astype(np.float16)
    return np.concatenate([hi, lo], axis=1)


def kernel(**inputs):
    x = np.asarray(inputs["x"], np.float32)
    betas = [float(v) for v in np.asarray(inputs["beta"], np.float32)]
    alphas = [float(v) for v in np.asarray(inputs["alpha"], np.float32)]

    nc = _get_nc(betas, alphas)

    xt = np.ascontiguousarray(x.T)
    xth = xt.astype(np.float16)
    xtl = (xt - xth.astype(np.float32)).astype(np.float16)
    masksT = [
        np.ascontiguousarray(
            _ctx_mask_host(
                l,
                np.asarray(inputs[f"cw{l}"], np.float32),
                np.asarray(inputs[f"asg{l}"]),
                x.shape[0],
            ).T
        ).astype(np.float16)
        for l in range(N_LAYERS)
    ]

    in_maps = []
    for c in range(NCORES):
        sl = slice(c * S, (c + 1) * S)
        m = {"xth": xth, "xtl": xtl}
        for l in range(N_LAYERS):
            wr = np.asarray(inputs[f"W{l}r"], np.float32)[:, sl]
            wi = np.asarray(inputs[f"W{l}i"], np.float32)[:, sl]
            m[f"w{l}rp"] = _split16(wr)
            m[f"w{l}ip"] = _split16(wi)
            m[f"mask{l}"] = masksT[l][sl, :]
        in_maps.append(m)

    res = run_bass_kernel_spmd(nc, in_maps, core_ids=list(range(NCORES)),
                               trace=TRACE)
    global LAST_RES
    LAST_RES = res
    outr = np.concatenate([res.results[c]["outr"] for c in range(NCORES)], axis=0)
    outi = np.concatenate([res.results[c]["outi"] for c in range(NCORES)], axis=0)
    return (outr.T + 1j * outi.T).astype(np.complex64)


# revision 24
# speedup vs baseline: 1.1523x; 1.0023x over previous
"""Trainium2 Bass kernel for nn_BioinspiredNeuralNetwork (3-layer holographic MLP).

Math per layer i (complex):
    out = xc @ (Wr + i*Wi)
    act = sigmoid(beta_i * out.real) ** alpha_i
    xc  = act * out / |out| * mask_i     (mask: fixed PRNG key 42, host-precomputed)

Distribution: tensor-parallel over output columns across 8 cores (512 cols
each). Activations kept transposed [4096, 512] (neuron-major), replicated via
AllGather at each layer boundary.

Precision: 3-pass fp16 split matmuls (x = x_hi + x_lo, W = W_hi + W_lo, drop
lo*lo) — fp32-class error at full 1-cycle/row PE rate (HW-verified rms rel
err 2.6e-7 per 4096-deep matmul). The complex real part uses two PSUM banks
(P1 = xr@Wr, P2 = xi@Wi) combined as P1 - P2 in the epilogue.

Exchange: activations cross the AllGather already split into fp16 hi/lo
pieces (split once on the producing core — fp16 ALU casts are slow on DVE, so
receive-side splitting of all 32 chunks is avoided; reloads are pure DMAs
into the resident piece tiles). The input x is pre-split on the host.

Overlap: each layer's 4 output m-tiles run in two phases (m01, m23). The
boundary AllGather is split in two halves: half A launches after phase 1's
epilogue and hides under phase 2's matmuls; half B hides under the next
layer's first-half chunks. All k-loops process half-A chunks first so
boundary reloads can overwrite them early.
"""

import numpy as np

import concourse.bass as bass
import concourse.mybir as mybir
import concourse.tile as tile
from concourse import bacc
from concourse.bass import ds, ts
from concourse.bass_utils import run_bass_kernel_spmd

AF = mybir.ActivationFunctionType

NCORES = 8
B = 512            # batch
D = 4096           # layer width
S = D // NCORES    # per-core output column shard (512)
KT = D // 128      # 32 k-chunks
MT = S // 128      # 4 m-tiles per shard
N_LAYERS = 3
N_CLUSTERS = 10
SPARSITY = 0.2

f32 = mybir.dt.float32
f16 = mybir.dt.float16

# The boundary AllGather is split in four quarters, one per m-tile: quarter q
# carries shard rows q*128:(q+1)*128 of every core, i.e. chunks {4c+q}. Every
# k-loop processes chunks in quarter order so reload DMAs can overwrite the
# earliest-read chunks while later quarters are still gathering.
_CHUNKS = [4 * c + q for q in range(4) for c in range(NCORES)]


def _build(betas, alphas):
    nc = bacc.Bacc("TRN2", target_bir_lowering=False, debug=False,
                   num_devices=NCORES)

    # input x^T pre-split into fp16 hi/lo pieces on the host
    xth = nc.dram_tensor("xth", [D, B], f16, kind="ExternalInput")
    xtl = nc.dram_tensor("xtl", [D, B], f16, kind="ExternalInput")
    # weight piece packs: [D, 2*S] fp16, cols 0:S = hi piece, S:2S = lo piece
    wrp = [nc.dram_tensor(f"w{l}rp", [D, 2 * S], f16, kind="ExternalInput")
           for l in range(N_LAYERS)]
    wip = [nc.dram_tensor(f"w{l}ip", [D, 2 * S], f16, kind="ExternalInput")
           for l in range(N_LAYERS)]
    msk = [nc.dram_tensor(f"mask{l}", [S, B], f16, kind="ExternalInput")
           for l in range(N_LAYERS)]
    outr = nc.dram_tensor("outr", [S, B], f32, kind="ExternalOutput")
    outi = nc.dram_tensor("outi", [S, B], f32, kind="ExternalOutput")

    # exchange buffers per boundary/quarter, fp16 pieces:
    # rows [xrh(128); xrl(128); xih(128); xil(128)]
    agi = [[nc.dram_tensor(f"agi{b}{q}", [S, B], f16) for q in range(4)]
           for b in range(2)]
    ago = [[nc.dram_tensor(f"ago{b}{q}", [NCORES * S, B], f16,
                           addr_space="Shared") for q in range(4)]
           for b in range(2)]

    xth_r = xth.ap().rearrange("(n p) b -> n p b", p=128)
    xtl_r = xtl.ap().rearrange("(n p) b -> n p b", p=128)
    wrp_r = [w.ap().rearrange("(n p) s -> n p s", p=128) for w in wrp]
    wip_r = [w.ap().rearrange("(n p) s -> n p s", p=128) for w in wip]
    msk_r = [m.ap().rearrange("(n p) b -> n p b", p=128) for m in msk]

    with tile.TileContext(nc) as tc:
        with (
            tc.tile_pool(name="xp", bufs=1) as xp,
            tc.tile_pool(name="wp", bufs=4) as wp,
            tc.tile_pool(name="mp", bufs=1) as mp,
            tc.tile_pool(name="ep", bufs=1) as ep,
            tc.tile_pool(name="ps", bufs=1, space="PSUM") as ps,
        ):
            # resident fp16 hi/lo pieces of transposed activations
            xrh = [xp.tile([128, B], f16, name=f"xrh{k}", tag=f"xrh{k}")
                   for k in range(KT)]
            xrl = [xp.tile([128, B], f16, name=f"xrl{k}", tag=f"xrl{k}")
                   for k in range(KT)]
            xih = [xp.tile([128, B], f16, name=f"xih{k}", tag=f"xih{k}")
                   for k in range(KT)]
            xil = [xp.tile([128, B], f16, name=f"xil{k}", tag=f"xil{k}")
                   for k in range(KT)]

            def mm_phase(l, pair, scope, load_x=False):
                """One m-pair phase of layer l's matmuls -> psum dict per m.
                l == 0: p1 = x@Wr, pi = x@Wi (real input).
                l >= 1: p1 = xr@Wr, p2 = xi@Wi, pi = xr@Wi + xi@Wr."""
                with nc.named_scope(scope):
                    p1 = {m: ps.tile([128, B], f32, name=f"p1s{i}", tag=f"p1s{i}")
                          for i, m in enumerate(pair)}
                    pi = {m: ps.tile([128, B], f32, name=f"pis{i}", tag=f"pis{i}")
                          for i, m in enumerate(pair)}
                    p2 = ({m: ps.tile([128, B], f32, name=f"p2s{i}",
                                      tag=f"p2s{i}") for i, m in enumerate(pair)}
                          if l > 0 else None)
                    for ki, k in enumerate(_CHUNKS):
                        first, last = ki == 0, ki == KT - 1
                        if load_x:
                            nc.sync.dma_start(out=xrh[k][:], in_=xth_r[k])
                            nc.sync.dma_start(out=xrl[k][:], in_=xtl_r[k])
                        wr_t = wp.tile([128, 2 * S], f16, name="wr_t", tag="wr_t")
                        nc.sync.dma_start(out=wr_t[:], in_=wrp_r[l][k])
                        wi_t = wp.tile([128, 2 * S], f16, name="wi_t", tag="wi_t")
                        nc.sync.dma_start(out=wi_t[:], in_=wip_r[l][k])
                        mm = nc.tensor.matmul

                        def passes(p, wt, m, rh_t, rl_t, first, last):
                            hi = wt[:, ts(m, 128)]
                            lo = wt[:, ds(S + m * 128, 128)]
                            mm(p[:], lhsT=hi, rhs=rh_t[:], start=first,
                               stop=False)
                            mm(p[:], lhsT=hi, rhs=rl_t[:], start=False,
                               stop=False)
                            mm(p[:], lhsT=lo, rhs=rh_t[:], start=False,
                               stop=last)

                        for m in pair:
                            passes(p1[m], wr_t, m, xrh[k], xrl[k],
                                   first, last)
                            if l == 0:
                                passes(pi[m], wi_t, m, xrh[k], xrl[k],
                                       first, last)
                            else:
                                passes(p2[m], wi_t, m, xih[k], xil[k],
                                       first, last)
                                passes(pi[m], wi_t, m, xrh[k], xrl[k],
                                       first, False)
                                passes(pi[m], wr_t, m, xih[k], xil[k],
                                       False, last)
                    return p1, p2, pi

            def epilogue(l, pair, p1, p2, pi, mt, scope):
                """l == 0: out_r = p1, out_i = pi.  l >= 1: out_r = p1 - p2.
                For boundary layers, split xnr/xni into fp16 pieces here
                (producer-side) and stage them for the AllGather."""
                with nc.named_scope(scope):
                    for m in pair:
                        if l == 0:
                            orr_ap, oi_ap = p1[m][:], pi[m][:]
                        else:
                            c2 = ep.tile([128, B], f32, name="c2", tag="c2")
                            nc.scalar.copy(c2[:], p2[m][:])
                            orr = ep.tile([128, B], f32, name="orr", tag="orr")
                            nc.vector.tensor_sub(orr[:], p1[m][:], c2[:])
                            orr_ap, oi_ap = orr[:], pi[m][:]
                        act = ep.tile([128, B], f32, name="act", tag="act", bufs=2)
                        nc.scalar.activation(act[:], orr_ap, AF.Sigmoid,
                                             scale=float(betas[l]))
                        if abs(alphas[l] - 1.0) > 1e-12:
                            lg = ep.tile([128, B], f32, name="lg", tag="lg")
                            nc.scalar.activation(lg[:], act[:], AF.Ln)
                            nc.scalar.activation(act[:], lg[:], AF.Exp,
                                                 scale=float(alphas[l]))
                        u1 = ep.tile([128, B], f32, name="u1", tag="u1", bufs=2)
                        nc.scalar.activation(u1[:], orr_ap, AF.Square)
                        u2 = ep.tile([128, B], f32, name="u2", tag="u2", bufs=2)
                        nc.scalar.activation(u2[:], oi_ap, AF.Square)
                        u3 = ep.tile([128, B], f32, name="u3", tag="u3", bufs=2)
                        nc.vector.tensor_add(u3[:], u1[:], u2[:])
                        rin = ep.tile([128, B], f32, name="u1b", tag="u1b", bufs=2)
                        nc.vector.reciprocal(rin[:], u3[:])
                        q = ep.tile([128, B], f32, name="u2b", tag="u2b", bufs=2)
                        nc.scalar.activation(q[:], rin[:], AF.Sqrt)
                        f = ep.tile([128, B], f32, name="u3b", tag="u3b", bufs=2)
                        nc.vector.tensor_mul(f[:], act[:], q[:])
                        fm = ep.tile([128, B], f32, name="fm", tag="fm", bufs=2)
                        nc.vector.tensor_mul(fm[:], f[:], mt[m][:])
                        xnr = ep.tile([128, B], f32, name="xnr", tag="xnr", bufs=2)
                        nc.vector.tensor_mul(xnr[:], fm[:], orr_ap)
                        xni = ep.tile([128, B], f32, name="xni", tag="xni", bufs=2)
                        nc.vector.tensor_mul(xni[:], fm[:], oi_ap)
                        if l == N_LAYERS - 1:
                            nc.sync.dma_start(out=outr.ap()[ts(m, 128)],
                                              in_=xnr[:])
                            nc.sync.dma_start(out=outi.ap()[ts(m, 128)],
                                              in_=xni[:])
                        else:
                            # producer-side fp16 split (DVE for r, GpSimd for i)
                            nrh = ep.tile([128, B], f16, name="nrh", tag="nrh")
                            nc.vector.tensor_copy(nrh[:], xnr[:])
                            nrl = ep.tile([128, B], f16, name="nrl", tag="nrl")
                            nc.vector.tensor_sub(nrl[:], xnr[:], nrh[:])
                            nih = ep.tile([128, B], f16, name="nih", tag="nih")
                            nc.gpsimd.tensor_copy(nih[:], xni[:])
                            nil_ = ep.tile([128, B], f16, name="nil", tag="nil")
                            nc.gpsimd.tensor_sub(nil_[:], xni[:], nih[:])
                            dst = agi[l][m].ap()
                            for sec, t in enumerate((nrh, nrl, nih, nil_)):
                                nc.sync.dma_start(
                                    out=dst[ds(sec * 128, 128)], in_=t[:])
                            with nc.named_scope(f"x{l}q{m}"):
                                allgather(l, m)

            def allgather(l, q):
                nc.gpsimd.collective_compute(
                    "AllGather", mybir.AluOpType.bypass,
                    ins=[agi[l][q].ap().opt()],
                    outs=[ago[l][q].ap().opt()],
                    replica_groups=[list(range(NCORES))],
                )

            def reload_quarter(l, q, scope):
                """Pure DMAs: gathered fp16 pieces -> resident piece tiles."""
                with nc.named_scope(scope):
                    gao = ago[l][q].ap()
                    for c in range(NCORES):
                        k = 4 * c + q
                        base = c * S
                        for sec, tiles in enumerate((xrh, xrl, xih, xil)):
                            nc.sync.dma_start(
                                out=tiles[k][:],
                                in_=gao[ds(base + sec * 128, 128)])

            for l in range(N_LAYERS):
                mt = [mp.tile([128, B], f16, name=f"mt{m}", tag=f"mt{m}")
                      for m in range(MT)]
                for m in range(MT):
                    nc.sync.dma_start(out=mt[m][:], in_=msk_r[l][m])

                p1, p2, pi = mm_phase(l, (0, 1), f"l{l}p1", load_x=(l == 0))
                epilogue(l, (0, 1), p1, p2, pi, mt, f"l{l}e1")
                p1, p2, pi = mm_phase(l, (2, 3), f"l{l}p2")
                epilogue(l, (2, 3), p1, p2, pi, mt, f"l{l}e2")
                if l < N_LAYERS - 1:
                    for q in range(4):
                        reload_quarter(l, q, f"r{l}q{q}")

    nc.compile()
    return nc


_NC_CACHE: dict = {}
TRACE = False
LAST_RES = None


def _get_nc(betas, alphas):
    key = (tuple(betas), tuple(alphas))
    if key not in _NC_CACHE:
        _NC_CACHE[key] = _build(betas, alphas)
    return _NC_CACHE[key]


def _ctx_mask_host(layer_i, cw, asg, batch):
    """Exact replica of reference._ctx_mask — fixed PRNG key, depends on
    inputs only through cw (cluster weights) and asg (cluster assignment)."""
    import jax
    import jax.numpy as jnp

    cpu = jax.devices("cpu")[0]
    with jax.default_device(cpu):
        key = jax.random.fold_in(jax.random.key(42), layer_i)
        cw_j = jnp.asarray(cw)
        asg_j = jnp.asarray(asg)
        probs = jax.nn.softmax(cw_j)
        p = probs[asg_j] * SPARSITY
        n = asg.shape[0]
        k1, k2 = jax.random.split(key)
        bern = jax.random.uniform(k1, (batch, n)) < p
        u = jax.random.uniform(k2, (batch, n))
        segmax = jax.vmap(
            lambda ur: jax.ops.segment_max(ur, asg_j, num_segments=N_CLUSTERS)
        )(u)
        force = u >= segmax[:, asg_j]
        return np.asarray((bern | force).astype(jnp.float32))


def _split16(w):
    hi = w.astype(np.float16)
    lo = (w - hi.astype(np.float32)).astype(np.float16)
    return np.concatenate([hi, lo], axis=1)


def kernel(**inputs):
    x = np.asarray(inputs["x"], np.float32)
    betas = [float(v) for v in np.asarray(inputs["beta"], np.float32)]
    alphas = [float(v) for v in np.asarray(inputs["alpha"], np.float32)]

    nc = _get_nc(betas, alphas)

    xt = np.ascontiguousarray(x.T)
    xth = xt.astype(np.float16)
    xtl = (xt - xth.astype(np.float32)).astype(np.float16)
    masksT = [
        np.ascontiguousarray(
            _ctx_mask_host(
                l,
                np.asarray(inputs[f"cw{l}"], np.float32),
                np.asarray(inputs[f"asg{l}"]),
                x.shape[0],
            ).T
        ).astype(np.float16)
        for l in range(N_LAYERS)
    ]

    in_maps = []
    for c in range(NCORES):
        sl = slice(c * S, (c + 1) * S)
        m = {"xth": xth, "xtl": xtl}
        for l in range(N_LAYERS):
            wr = np.asarray(inputs[f"W{l}r"], np.float32)[:, sl]
            wi = np.asarray(inputs[f"W{l}i"], np.float32)[:, sl]
            m[f"w{l}rp"] = _split16(wr)
            m[f"w{l}ip"] = _split16(wi)
            m[f"mask{l}"] = masksT[l][sl, :]
        in_maps.append(m)

    res = run_bass_kernel_spmd(nc, in_maps, core_ids=list(range(NCORES)),
                               trace=TRACE)
    global LAST_RES
    LAST_RES = res
    outr = np.concatenate([res.results[c]["outr"] for c in range(NCORES)], axis=0)
    outi = np.concatenate([res.results[c]["outi"] for c in range(NCORES)], axis=0)
    return (outr.T + 1j * outi.T).astype(np.complex64)
